# revision 7
# baseline (speedup 1.0000x reference)
"""Trainium2 Bass kernel for a single nGPT-style attention head.

Computation (see reference): fused QKV projection, RoPE over the full head
dim, L2-normalize q/k scaled by sqk, causal SDPA with scale sqrt(d_model).

Sharding: data-parallel over batch — 8 batch elements, one per NeuronCore.

v2 design notes (vs the GPSIMD-heavy v1):
  - No GPSIMD compute. rotate_half runs as two SBUF->SBUF partition-shift
    DMAs on the merged q|k tile; the 1/||q|| partition broadcast is a
    DRAM-roundtrip broadcast DMA (stride-0 partition read from DRAM).
  - 1/||k|| never gets broadcast: it rides into the attention exp as the
    per-partition activation scale (scores strips are [tk, tq], tk on
    partitions). It is computed directly in [tk, tile] layout by making
    the squared-q/k chunk the matmul *stationary* and ones the moving
    operand (out [t-chunk, 1] lands t-on-partitions), so the Ln/Exp
    chain runs on [128, 8] tiles instead of 1-lane [1, 512] rows.
  - All ACT functions stay in the natural_log_exp_and_others table set
    (copy/ln/exp) — the activation table loads exactly once.
  - Causal masking streams only the valid column suffix of each strip
    (no zero-fill DMAs); the 128x128 diagonal tile gets a DVE tri-mask.
  - Softmax denominator reciprocal via Ln/Exp(-1); its partition
    broadcast is a K=1 ones-row matmul into PSUM (cheap on PE).
  - Bulk DMAs (v roundtrip, output) issue from the otherwise-idle GPSIMD
    queue (SWDGE) to keep the SP sequencer (565ns/DMA) off the critical
    path.
"""

import numpy as np
import ml_dtypes

import concourse.bass as bass
import concourse.tile as tile
from concourse import bacc, mybir
from concourse.bass import ts, ds
from concourse.bass_utils import run_bass_kernel_spmd

# Surface compile-hook exceptions (the PJRT bridge swallows tracebacks).
try:
    import traceback
    import libneuronxla as _lnx

    if not getattr(_lnx, "_err_wrapped", False):
        _orig_cc = _lnx.neuronx_cc

        def _cc_wrapper(*a, **kw):
            try:
                return _orig_cc(*a, **kw)
            except BaseException:
                traceback.print_exc()
                raise

        _lnx.neuronx_cc = _cc_wrapper
        _lnx._err_wrapped = True
except Exception:
    pass

AFT = mybir.ActivationFunctionType
ALU = mybir.AluOpType
F32 = mybir.dt.float32
BF16 = mybir.dt.bfloat16

B, T_FULL, C, D = 8, 2048, 1024, 128
ROPE_BASE = 10000.0
P = 128
TB = 512  # t-block (tq block width, PSUM-bank free dim)
NCO = C // P  # contraction chunks for the QKV projection
H = P // 2


def build_nc(T=T_FULL, num_devices=8):
    from contextlib import ExitStack
    NTB = T // TB
    NKT = T // P
    NC = TB // P  # 128-chunks per block
    nc = bacc.Bacc("TRN2", target_bir_lowering=False, debug=False,
                   num_devices=num_devices)

    xT = nc.dram_tensor("xT", [C, T], BF16, kind="ExternalInput").ap()
    WT = nc.dram_tensor("WT", [C, 3 * D], BF16, kind="ExternalInput").ap()
    cosF = nc.dram_tensor("cosF", [P, T], BF16, kind="ExternalInput").ap()
    sinF = nc.dram_tensor("sinF", [P, T], BF16, kind="ExternalInput").ap()
    tri = nc.dram_tensor("tri", [P, P], BF16, kind="ExternalInput").ap()
    sqk = nc.dram_tensor("sqk", [D, 1], F32, kind="ExternalInput").ap()
    onb = nc.dram_tensor("onb", [P, 1], BF16, kind="ExternalInput").ap()
    onr = nc.dram_tensor("onr", [1, P], BF16, kind="ExternalInput").ap()
    outT = nc.dram_tensor("outT", [D, T], BF16, kind="ExternalOutput").ap()

    xT_t = xT.rearrange("(co p) t -> p co t", p=P)
    WT_t = WT.rearrange("(co p) d -> p co d", p=P)

    with tile.TileContext(nc) as tc:
        with ExitStack() as ctx:
            const = ctx.enter_context(tc.tile_pool(name="const", bufs=1))
            wpool = ctx.enter_context(tc.tile_pool(name="wpool", bufs=3))
            dramp = ctx.enter_context(
                tc.tile_pool(name="dramp", bufs=1, space="DRAM"))

            wt = const.tile([P, NCO, 3 * D], BF16)
            nc.sync.dma_start(wt, WT_t)
            sqk_sb = const.tile([D, 1], F32)
            nc.sync.dma_start(sqk_sb, sqk)
            ones_k = const.tile([P, 1], BF16)
            nc.sync.dma_start(ones_k, onb)
            ones_r = const.tile([1, P], BF16)
            nc.sync.dma_start(ones_r, onr)
            tri_sb = const.tile([P, P], BF16)
            nc.sync.dma_start(tri_sb, tri)
            cos_sb = const.tile([P, T], BF16)
            nc.sync.dma_start(cos_sb, cosF)
            sin_sb = const.tile([P, T], BF16)
            nc.sync.dma_start(sin_sb, sinF)
            # (sqk * C^(1/4))^2 = sqrt(C) * sqk^2 — full logit scale, on q.
            sqk232 = const.tile([D, 1], F32)
            nc.vector.tensor_scalar_mul(sqk232, sqk_sb, float(C ** 0.25))
            nc.vector.tensor_mul(sqk232, sqk232, sqk232)

            qk = const.tile([P, 2 * T], BF16)   # q̃^T | k̃^T (k unnormalized)
            vt = const.tile([P, NKT, P], BF16)  # v tiles [tk, e]
            ink = const.tile([P, NKT], F32)     # 1/||k|| as [tk%P, tile]
            vd = dramp.tile([P, T], BF16)
            invq_d = dramp.tile([1, T], BF16)   # 1/||q|| row for broadcast
            invd_d = dramp.tile([1, T], BF16)   # 1/denom row for broadcast

            # ---------- Phase A: QKV + norms + RoPE (per block) ----------
            with ExitStack() as actx:
                xpool = actx.enter_context(tc.tile_pool(name="xpool", bufs=2))
                ps_qkv = actx.enter_context(
                    tc.tile_pool(name="ps_qkv", bufs=2, space="PSUM"))
                ps_n = actx.enter_context(
                    tc.tile_pool(name="ps_n", bufs=2, space="PSUM"))
                for j in range(NTB):
                    tsl = ds(j * TB, TB)
                    with nc.named_scope(f"qkv{j}"):
                        xts = xpool.tile([P, NCO, TB], BF16, tag="xt")
                        nc.sync.dma_start(xts, xT_t[:, :, tsl])
                        ps = ps_qkv.tile([P, 3, TB], F32, tag="qkv")
                        for g in range(3):
                            for co in range(NCO):
                                nc.tensor.matmul(
                                    ps[:, g, :], wt[:, co, ts(g, D)],
                                    xts[:, co, :],
                                    start=(co == 0), stop=(co == NCO - 1))
                        qkraw = wpool.tile([P, 2, TB], BF16, tag="qkraw")
                        nc.scalar.activation(qkraw[:, 0, :], ps[:, 0, :],
                                             AFT.Copy)
                        nc.scalar.activation(qkraw[:, 1, :], ps[:, 1, :],
                                             AFT.Copy)
                        vst = wpool.tile([P, TB], BF16, tag="vst")
                        nc.vector.tensor_copy(vst, ps[:, 2, :])

                    with nc.named_scope(f"norm{j}"):
                        # ||.||^2 per column, t-on-partitions: squared q/k
                        # chunk as stationary, ones as moving — out [128,1]
                        # per chunk. One [P, 2*NC] Ln/Exp serves the block.
                        sq = wpool.tile([P, 2, TB], BF16, tag="sq")
                        nc.vector.tensor_mul(sq[:, 0, :], qkraw[:, 0, :],
                                             qkraw[:, 0, :])
                        nc.vector.tensor_mul(sq[:, 1, :], qkraw[:, 1, :],
                                             qkraw[:, 1, :])
                        nps = ps_n.tile([P, 2 * NC], F32, tag="n")
                        for c in range(NC):
                            nc.tensor.matmul(
                                nps[:, ds(c, 1)], sq[:, 0, ts(c, P)],
                                ones_k, start=True, stop=True)
                            nc.tensor.matmul(
                                nps[:, ds(NC + c, 1)], sq[:, 1, ts(c, P)],
                                ones_k, start=True, stop=True)
                        lnn = wpool.tile([P, 2 * NC], F32, tag="lnn")
                        nc.scalar.activation(lnn, nps, AFT.Ln)
                        invq = wpool.tile([P, NC], BF16, tag="invq")
                        nc.scalar.activation(invq, lnn[:, 0:NC],
                                             AFT.Exp, scale=-0.5)
                        nc.scalar.activation(ink[:, ds(j * NC, NC)],
                                             lnn[:, NC:2 * NC],
                                             AFT.Exp, scale=-0.5)
                        # 1/||q|| to DRAM in row layout, broadcast back.
                        nc.sync.dma_start(
                            invq_d[0:1, tsl].rearrange(
                                "a (c p) -> (a p) c", p=P), invq)
                        bcq = wpool.tile([P, TB], BF16, tag="bcq")
                        nc.sync.dma_start(
                            bcq, invq_d[0:1, tsl].broadcast_to([P, TB]))

                    with nc.named_scope(f"rope{j}"):
                        # rotate_half via partition-shift DMAs (sign folded
                        # into the sin table); q and k in one shot.
                        rot = wpool.tile([P, 2, TB], BF16, tag="rot")
                        nc.sync.dma_start(rot[0:H, :, :], qkraw[H:P, :, :])
                        nc.sync.dma_start(rot[H:P, :, :], qkraw[0:H, :, :])

                        m1 = wpool.tile([P, TB], BF16, tag="m1")
                        nc.vector.tensor_mul(m1, qkraw[:, 0, :],
                                             cos_sb[:, tsl])
                        m2 = wpool.tile([P, TB], BF16, tag="m2")
                        nc.vector.tensor_mul(m2, rot[:, 0, :],
                                             sin_sb[:, tsl])
                        m12 = wpool.tile([P, TB], BF16, tag="m12")
                        nc.vector.tensor_add(m12, m1, m2)
                        nc.vector.scalar_tensor_tensor(
                            out=qk[:, tsl], in0=m12, scalar=sqk232,
                            in1=bcq, op0=ALU.mult, op1=ALU.mult)

                        m1k = wpool.tile([P, TB], BF16, tag="m1k")
                        nc.vector.tensor_mul(m1k, qkraw[:, 1, :],
                                             cos_sb[:, tsl])
                        m2k = wpool.tile([P, TB], BF16, tag="m2k")
                        nc.vector.tensor_mul(m2k, rot[:, 1, :],
                                             sin_sb[:, tsl])
                        nc.vector.tensor_add(qk[:, ds(T + j * TB, TB)],
                                             m1k, m2k)

                        # v transpose via DRAM-roundtrip XBAR DMA (bf16)
                        nc.gpsimd.dma_start(vd[:, tsl], vst)
                        for i in range(4 * j, 4 * j + 4):
                            nc.sync.dma_start_transpose(vt[:, i, :],
                                                        vd[:, ts(i, P)])

            # ---------- Phase C: causal attention ----------
            with ExitStack() as cctx:
                expool = cctx.enter_context(
                    tc.tile_pool(name="expool", bufs=4))
                ps_sc = cctx.enter_context(
                    tc.tile_pool(name="ps_sc", bufs=4, space="PSUM"))
                ps_o = cctx.enter_context(
                    tc.tile_pool(name="ps_o", bufs=2, space="PSUM"))
                ps_d = cctx.enter_context(
                    tc.tile_pool(name="ps_d", bufs=2, space="PSUM"))

                for J in range(NTB):
                    with nc.named_scope(f"att{J}"):
                        q_blk = qk[:, ts(J, TB)]
                        po = ps_o.tile([P, TB], F32, tag="o")
                        pd = ps_d.tile([1, TB], F32, tag="d")
                        nstr = (TB // P) * (J + 1)
                        for i in range(nstr):
                            dr = i - (TB // P) * J
                            off = P * dr if dr >= 0 else 0
                            w = TB - off
                            sc = ps_sc.tile([P, TB], F32, tag="sc")
                            nc.tensor.matmul(
                                sc[:, ds(off, w)], qk[:, ds(T + P * i, P)],
                                q_blk[:, ds(off, w)], start=True, stop=True)
                            ex = expool.tile([P, TB], BF16, tag="ex")
                            nc.scalar.activation(
                                ex[:, ds(off, w)], sc[:, ds(off, w)],
                                AFT.Exp, scale=ink[:, i:i + 1])
                            if dr >= 0:
                                nc.vector.tensor_mul(
                                    ex[:, ds(off, P)], ex[:, ds(off, P)],
                                    tri_sb)
                            nc.tensor.matmul(
                                po[:, ds(off, w)], vt[:, i, :],
                                ex[:, ds(off, w)],
                                start=(i == 0), stop=(i == nstr - 1))
                            nc.tensor.matmul(
                                pd[:, ds(off, w)], ones_k,
                                ex[:, ds(off, w)],
                                start=(i == 0), stop=(i == nstr - 1))

                        lnd = wpool.tile([1, TB], F32, tag="lnd")
                        nc.scalar.activation(lnd, pd, AFT.Ln)
                        invd = wpool.tile([1, TB], BF16, tag="invd")
                        nc.scalar.activation(invd, lnd, AFT.Exp, scale=-1.0)
                        nc.sync.dma_start(invd_d[0:1, ts(J, TB)], invd)
                        bcd = wpool.tile([P, TB], BF16, tag="bcd")
                        nc.sync.dma_start(
                            bcd,
                            invd_d[0:1, ts(J, TB)].broadcast_to([P, TB]))
                        ob = wpool.tile([P, TB], BF16, tag="ob")
                        nc.vector.tensor_mul(ob, po, bcd)
                        nc.gpsimd.dma_start(outT[:, ts(J, TB)], ob)

    nc.compile()
    return nc


def _host_tables(T):
    d = D
    inv_freq = 1.0 / (ROPE_BASE ** (np.arange(0, d, 2, dtype=np.float64) / d))
    t = np.arange(T, dtype=np.float64)
    freqs = np.outer(inv_freq, t)  # [d/2, T]
    emb = np.concatenate([freqs, freqs], axis=0)  # [d, T]
    cos1 = np.cos(emb)
    sin1 = np.sin(emb)
    # sign of rotate_half folded into the table: rot is built with plain
    # copies, and sin rows 0:d/2 carry the minus sign instead.
    sin1[: d // 2, :] *= -1.0
    cosF = np.ascontiguousarray(cos1).astype(ml_dtypes.bfloat16)
    sinF = np.ascontiguousarray(sin1).astype(ml_dtypes.bfloat16)
    a = np.arange(P)
    tri = (a[None, :] >= a[:, None]).astype(ml_dtypes.bfloat16)  # [tk, tq]
    return cosF, sinF, tri


TRACE = False
LAST_EXEC_NS = None
LAST_TRACE = None
LAST_INSTS = None


def kernel(x, W_qkv, sqk):
    global LAST_EXEC_NS, LAST_TRACE, LAST_INSTS
    T = x.shape[1]
    cosF, sinF, tri = _host_tables(T)
    WT = np.ascontiguousarray(np.asarray(W_qkv).T).astype(ml_dtypes.bfloat16)
    sqk2 = np.ascontiguousarray(
        np.asarray(sqk).reshape(D, 1)).astype(np.float32)
    in_maps = []
    for b in range(B):
        in_maps.append({
            "xT": np.ascontiguousarray(
                np.asarray(x[b]).T).astype(ml_dtypes.bfloat16),
            "WT": WT,
            "cosF": cosF,
            "sinF": sinF,
            "tri": tri,
            "sqk": sqk2,
            "onb": np.ones((P, 1), ml_dtypes.bfloat16),
            "onr": np.ones((1, P), ml_dtypes.bfloat16),
        })
    nc = build_nc(T=T, num_devices=B)
    res = run_bass_kernel_spmd(nc, in_maps, core_ids=list(range(B)),
                               trace=TRACE)
    LAST_EXEC_NS = res.exec_time_ns
    LAST_TRACE = (res.instructions_and_trace[1]
                  if res.instructions_and_trace else None)
    LAST_INSTS = (res.instructions_and_trace[0]
                  if res.instructions_and_trace else None)
    out = np.stack([r["outT"].T for r in res.results])  # [B, T, D]
    return np.ascontiguousarray(out).astype(np.float32)


# revision 8
# speedup vs baseline: 1.1389x; 1.1389x over previous
"""Trainium2 Bass kernel for a single nGPT-style attention head.

Computation (see reference): fused QKV projection, RoPE over the full head
dim, L2-normalize q/k scaled by sqk, causal SDPA with scale sqrt(d_model).

Sharding: data-parallel over batch — 8 batch elements, one per NeuronCore.

v2 design notes (vs the GPSIMD-heavy v1):
  - No GPSIMD compute. rotate_half runs as two SBUF->SBUF partition-shift
    DMAs on the merged q|k tile; the 1/||q|| partition broadcast is a
    DRAM-roundtrip broadcast DMA (stride-0 partition read from DRAM).
  - 1/||k|| never gets broadcast: it rides into the attention exp as the
    per-partition activation scale (scores strips are [tk, tq], tk on
    partitions). It is computed directly in [tk, tile] layout by making
    the squared-q/k chunk the matmul *stationary* and ones the moving
    operand (out [t-chunk, 1] lands t-on-partitions), so the Ln/Exp
    chain runs on [128, 8] tiles instead of 1-lane [1, 512] rows.
  - All ACT functions stay in the natural_log_exp_and_others table set
    (copy/ln/exp) — the activation table loads exactly once.
  - Causal masking streams only the valid column suffix of each strip
    (no zero-fill DMAs); the 128x128 diagonal tile gets a DVE tri-mask.
  - Softmax denominator reciprocal via Ln/Exp(-1); its partition
    broadcast is a K=1 ones-row matmul into PSUM (cheap on PE).
  - Bulk DMAs (v roundtrip, output) issue from the otherwise-idle GPSIMD
    queue (SWDGE) to keep the SP sequencer (565ns/DMA) off the critical
    path.
"""

import numpy as np
import ml_dtypes

import concourse.bass as bass
import concourse.tile as tile
from concourse import bacc, mybir
from concourse.bass import ts, ds
from concourse.bass_utils import run_bass_kernel_spmd

# Surface compile-hook exceptions (the PJRT bridge swallows tracebacks).
try:
    import traceback
    import libneuronxla as _lnx

    if not getattr(_lnx, "_err_wrapped", False):
        _orig_cc = _lnx.neuronx_cc

        def _cc_wrapper(*a, **kw):
            try:
                return _orig_cc(*a, **kw)
            except BaseException:
                traceback.print_exc()
                raise

        _lnx.neuronx_cc = _cc_wrapper
        _lnx._err_wrapped = True
except Exception:
    pass

AFT = mybir.ActivationFunctionType
ALU = mybir.AluOpType
F32 = mybir.dt.float32
BF16 = mybir.dt.bfloat16

B, T_FULL, C, D = 8, 2048, 1024, 128
ROPE_BASE = 10000.0
P = 128
TB = 512  # t-block (tq block width, PSUM-bank free dim)
NCO = C // P  # contraction chunks for the QKV projection
H = P // 2


def _pin_act_table():
    """Leave natural_log_exp_and_others as the only candidate activation
    table (it serves every func this kernel uses: copy/ln/exp/square), so
    the table-load pass emits exactly one ACT_TABLE_LOAD instead of
    thrashing 1.5us reloads on every ln<->exp alternation. Positions in
    the cached dict are untouched, so act_func_set_id stays a valid
    act_info.json index."""
    from concourse.hw_specs import get_activation_tables
    tabs = get_activation_tables("gen3")
    keep = "natural_log_exp_and_others"
    if keep in tabs:
        for name, funcs in tabs.items():
            if name != keep:
                funcs.clear()


def build_nc(T=T_FULL, num_devices=8):
    from contextlib import ExitStack
    _pin_act_table()
    NTB = T // TB
    NKT = T // P
    NC = TB // P  # 128-chunks per block
    nc = bacc.Bacc("TRN2", target_bir_lowering=False, debug=False,
                   num_devices=num_devices)

    xT = nc.dram_tensor("xT", [C, T], BF16, kind="ExternalInput").ap()
    WT = nc.dram_tensor("WT", [C, 3 * D], BF16, kind="ExternalInput").ap()
    cosF = nc.dram_tensor("cosF", [P, T], BF16, kind="ExternalInput").ap()
    sinF = nc.dram_tensor("sinF", [P, T], BF16, kind="ExternalInput").ap()
    tri = nc.dram_tensor("tri", [P, P], BF16, kind="ExternalInput").ap()
    sqk = nc.dram_tensor("sqk", [D, 1], F32, kind="ExternalInput").ap()
    onb = nc.dram_tensor("onb", [P, 1], BF16, kind="ExternalInput").ap()
    onr = nc.dram_tensor("onr", [1, P], BF16, kind="ExternalInput").ap()
    outT = nc.dram_tensor("outT", [D, T], BF16, kind="ExternalOutput").ap()

    xT_t = xT.rearrange("(co p) t -> p co t", p=P)
    WT_t = WT.rearrange("(co p) d -> p co d", p=P)

    with tile.TileContext(nc) as tc:
        with ExitStack() as ctx:
            const = ctx.enter_context(tc.tile_pool(name="const", bufs=1))
            wpool = ctx.enter_context(tc.tile_pool(name="wpool", bufs=3))
            dramp = ctx.enter_context(
                tc.tile_pool(name="dramp", bufs=1, space="DRAM"))

            wt = const.tile([P, NCO, 3 * D], BF16)
            nc.sync.dma_start(wt, WT_t)
            sqk_sb = const.tile([D, 1], F32)
            nc.sync.dma_start(sqk_sb, sqk)
            ones_k = const.tile([P, 1], BF16)
            nc.sync.dma_start(ones_k, onb)
            ones_r = const.tile([1, P], BF16)
            nc.sync.dma_start(ones_r, onr)
            tri_sb = const.tile([P, P], BF16)
            nc.sync.dma_start(tri_sb, tri)
            cos_sb = const.tile([P, T], BF16)
            nc.sync.dma_start(cos_sb, cosF)
            sin_sb = const.tile([P, T], BF16)
            nc.sync.dma_start(sin_sb, sinF)
            # (sqk * C^(1/4))^2 = sqrt(C) * sqk^2 — full logit scale, on q.
            sqk232 = const.tile([D, 1], F32)
            nc.vector.tensor_scalar_mul(sqk232, sqk_sb, float(C ** 0.25))
            nc.vector.tensor_mul(sqk232, sqk232, sqk232)

            qk = const.tile([P, 2 * T], BF16)   # q̃^T | k̃^T (k unnormalized)
            vt = const.tile([P, NKT, P], BF16)  # v tiles [tk, e]
            ink = const.tile([P, NKT], F32)     # 1/||k|| as [tk%P, tile]
            vd = dramp.tile([P, T], BF16)
            invq_d = dramp.tile([1, T], BF16)   # 1/||q|| row for broadcast
            invd_d = dramp.tile([1, T], BF16)   # 1/denom row for broadcast

            # ---------- Phase A: QKV + norms + RoPE (per block) ----------
            with ExitStack() as actx:
                xpool = actx.enter_context(tc.tile_pool(name="xpool", bufs=2))
                ps_qkv = actx.enter_context(
                    tc.tile_pool(name="ps_qkv", bufs=2, space="PSUM"))
                ps_n = actx.enter_context(
                    tc.tile_pool(name="ps_n", bufs=2, space="PSUM"))
                for j in range(NTB):
                    tsl = ds(j * TB, TB)
                    with nc.named_scope(f"qkv{j}"):
                        xts = xpool.tile([P, NCO, TB], BF16, tag="xt")
                        nc.sync.dma_start(xts, xT_t[:, :, tsl])
                        ps = ps_qkv.tile([P, 3, TB], F32, tag="qkv")
                        for g in range(3):
                            for co in range(NCO):
                                nc.tensor.matmul(
                                    ps[:, g, :], wt[:, co, ts(g, D)],
                                    xts[:, co, :],
                                    start=(co == 0), stop=(co == NCO - 1))
                        qkraw = wpool.tile([P, 2, TB], BF16, tag="qkraw")
                        nc.scalar.activation(qkraw[:, 0, :], ps[:, 0, :],
                                             AFT.Copy)
                        nc.scalar.activation(qkraw[:, 1, :], ps[:, 1, :],
                                             AFT.Copy)
                        vst = wpool.tile([P, TB], BF16, tag="vst")
                        nc.vector.tensor_copy(vst, ps[:, 2, :])

                    with nc.named_scope(f"norm{j}"):
                        # ||.||^2 per column, t-on-partitions: squared q/k
                        # chunk as stationary, ones as moving — out [128,1]
                        # per chunk. One [P, 2*NC] Ln/Exp serves the block.
                        sq = wpool.tile([P, 2, TB], BF16, tag="sq")
                        nc.vector.tensor_mul(sq[:, 0, :], qkraw[:, 0, :],
                                             qkraw[:, 0, :])
                        nc.vector.tensor_mul(sq[:, 1, :], qkraw[:, 1, :],
                                             qkraw[:, 1, :])
                        nps = ps_n.tile([P, 2 * NC], F32, tag="n")
                        for c in range(NC):
                            nc.tensor.matmul(
                                nps[:, ds(c, 1)], sq[:, 0, ts(c, P)],
                                ones_k, start=True, stop=True)
                            nc.tensor.matmul(
                                nps[:, ds(NC + c, 1)], sq[:, 1, ts(c, P)],
                                ones_k, start=True, stop=True)
                        lnn = wpool.tile([P, 2 * NC], F32, tag="lnn")
                        nc.scalar.activation(lnn, nps, AFT.Ln)
                        invq = wpool.tile([P, NC], BF16, tag="invq")
                        nc.scalar.activation(invq, lnn[:, 0:NC],
                                             AFT.Exp, scale=-0.5)
                        nc.scalar.activation(ink[:, ds(j * NC, NC)],
                                             lnn[:, NC:2 * NC],
                                             AFT.Exp, scale=-0.5)
                        # 1/||q|| to DRAM in row layout, broadcast back.
                        nc.sync.dma_start(
                            invq_d[0:1, tsl].rearrange(
                                "a (c p) -> (a p) c", p=P), invq)
                        bcq = wpool.tile([P, TB], BF16, tag="bcq")
                        nc.sync.dma_start(
                            bcq, invq_d[0:1, tsl].broadcast_to([P, TB]))

                    with nc.named_scope(f"rope{j}"):
                        # rotate_half via partition-shift DMAs (sign folded
                        # into the sin table); q and k in one shot.
                        rot = wpool.tile([P, 2, TB], BF16, tag="rot")
                        nc.sync.dma_start(rot[0:H, :, :], qkraw[H:P, :, :])
                        nc.sync.dma_start(rot[H:P, :, :], qkraw[0:H, :, :])

                        m1 = wpool.tile([P, TB], BF16, tag="m1")
                        nc.vector.tensor_mul(m1, qkraw[:, 0, :],
                                             cos_sb[:, tsl])
                        m2 = wpool.tile([P, TB], BF16, tag="m2")
                        nc.vector.tensor_mul(m2, rot[:, 0, :],
                                             sin_sb[:, tsl])
                        m12 = wpool.tile([P, TB], BF16, tag="m12")
                        nc.vector.tensor_add(m12, m1, m2)
                        nc.vector.scalar_tensor_tensor(
                            out=qk[:, tsl], in0=m12, scalar=sqk232,
                            in1=bcq, op0=ALU.mult, op1=ALU.mult)

                        m1k = wpool.tile([P, TB], BF16, tag="m1k")
                        nc.vector.tensor_mul(m1k, qkraw[:, 1, :],
                                             cos_sb[:, tsl])
                        m2k = wpool.tile([P, TB], BF16, tag="m2k")
                        nc.vector.tensor_mul(m2k, rot[:, 1, :],
                                             sin_sb[:, tsl])
                        nc.vector.tensor_add(qk[:, ds(T + j * TB, TB)],
                                             m1k, m2k)

                        # v transpose via DRAM-roundtrip XBAR DMA (bf16)
                        nc.gpsimd.dma_start(vd[:, tsl], vst)
                        for i in range(4 * j, 4 * j + 4):
                            nc.sync.dma_start_transpose(vt[:, i, :],
                                                        vd[:, ts(i, P)])

            # ---------- Phase C: causal attention ----------
            with ExitStack() as cctx:
                expool = cctx.enter_context(
                    tc.tile_pool(name="expool", bufs=4))
                ps_sc = cctx.enter_context(
                    tc.tile_pool(name="ps_sc", bufs=4, space="PSUM"))
                ps_o = cctx.enter_context(
                    tc.tile_pool(name="ps_o", bufs=2, space="PSUM"))
                ps_d = cctx.enter_context(
                    tc.tile_pool(name="ps_d", bufs=2, space="PSUM"))

                for J in range(NTB):
                    with nc.named_scope(f"att{J}"):
                        q_blk = qk[:, ts(J, TB)]
                        po = ps_o.tile([P, TB], F32, tag="o")
                        pd = ps_d.tile([1, TB], F32, tag="d")
                        nstr = (TB // P) * (J + 1)
                        for i in range(nstr):
                            dr = i - (TB // P) * J
                            off = P * dr if dr >= 0 else 0
                            w = TB - off
                            sc = ps_sc.tile([P, TB], F32, tag="sc")
                            nc.tensor.matmul(
                                sc[:, ds(off, w)], qk[:, ds(T + P * i, P)],
                                q_blk[:, ds(off, w)], start=True, stop=True)
                            ex = expool.tile([P, TB], BF16, tag="ex")
                            nc.scalar.activation(
                                ex[:, ds(off, w)], sc[:, ds(off, w)],
                                AFT.Exp, scale=ink[:, i:i + 1])
                            if dr >= 0:
                                nc.vector.tensor_mul(
                                    ex[:, ds(off, P)], ex[:, ds(off, P)],
                                    tri_sb)
                            nc.tensor.matmul(
                                po[:, ds(off, w)], vt[:, i, :],
                                ex[:, ds(off, w)],
                                start=(i == 0), stop=(i == nstr - 1))
                            nc.tensor.matmul(
                                pd[:, ds(off, w)], ones_k,
                                ex[:, ds(off, w)],
                                start=(i == 0), stop=(i == nstr - 1))

                        lnd = wpool.tile([1, TB], F32, tag="lnd")
                        nc.scalar.activation(lnd, pd, AFT.Ln)
                        invd = wpool.tile([1, TB], BF16, tag="invd")
                        nc.scalar.activation(invd, lnd, AFT.Exp, scale=-1.0)
                        nc.sync.dma_start(invd_d[0:1, ts(J, TB)], invd)
                        bcd = wpool.tile([P, TB], BF16, tag="bcd")
                        nc.sync.dma_start(
                            bcd,
                            invd_d[0:1, ts(J, TB)].broadcast_to([P, TB]))
                        ob = wpool.tile([P, TB], BF16, tag="ob")
                        nc.vector.tensor_mul(ob, po, bcd)
                        nc.gpsimd.dma_start(outT[:, ts(J, TB)], ob)

    nc.compile()
    return nc


def _host_tables(T):
    d = D
    inv_freq = 1.0 / (ROPE_BASE ** (np.arange(0, d, 2, dtype=np.float64) / d))
    t = np.arange(T, dtype=np.float64)
    freqs = np.outer(inv_freq, t)  # [d/2, T]
    emb = np.concatenate([freqs, freqs], axis=0)  # [d, T]
    cos1 = np.cos(emb)
    sin1 = np.sin(emb)
    # sign of rotate_half folded into the table: rot is built with plain
    # copies, and sin rows 0:d/2 carry the minus sign instead.
    sin1[: d // 2, :] *= -1.0
    cosF = np.ascontiguousarray(cos1).astype(ml_dtypes.bfloat16)
    sinF = np.ascontiguousarray(sin1).astype(ml_dtypes.bfloat16)
    a = np.arange(P)
    tri = (a[None, :] >= a[:, None]).astype(ml_dtypes.bfloat16)  # [tk, tq]
    return cosF, sinF, tri


TRACE = False
LAST_EXEC_NS = None
LAST_TRACE = None
LAST_INSTS = None


def kernel(x, W_qkv, sqk):
    global LAST_EXEC_NS, LAST_TRACE, LAST_INSTS
    T = x.shape[1]
    cosF, sinF, tri = _host_tables(T)
    WT = np.ascontiguousarray(np.asarray(W_qkv).T).astype(ml_dtypes.bfloat16)
    sqk2 = np.ascontiguousarray(
        np.asarray(sqk).reshape(D, 1)).astype(np.float32)
    in_maps = []
    for b in range(B):
        in_maps.append({
            "xT": np.ascontiguousarray(
                np.asarray(x[b]).T).astype(ml_dtypes.bfloat16),
            "WT": WT,
            "cosF": cosF,
            "sinF": sinF,
            "tri": tri,
            "sqk": sqk2,
            "onb": np.ones((P, 1), ml_dtypes.bfloat16),
            "onr": np.ones((1, P), ml_dtypes.bfloat16),
        })
    nc = build_nc(T=T, num_devices=B)
    res = run_bass_kernel_spmd(nc, in_maps, core_ids=list(range(B)),
                               trace=TRACE)
    LAST_EXEC_NS = res.exec_time_ns
    LAST_TRACE = (res.instructions_and_trace[1]
                  if res.instructions_and_trace else None)
    LAST_INSTS = (res.instructions_and_trace[0]
                  if res.instructions_and_trace else None)
    out = np.stack([r["outT"].T for r in res.results])  # [B, T, D]
    return np.ascontiguousarray(out).astype(np.float32)


# revision 9
# speedup vs baseline: 1.6474x; 1.4465x over previous
"""Trainium2 Bass kernel for a single nGPT-style attention head.

Computation (see reference): fused QKV projection, RoPE over the full head
dim, L2-normalize q/k scaled by sqk, causal SDPA with scale sqrt(d_model).

Sharding: data-parallel over batch — 8 batch elements, one per NeuronCore.

v3 design notes:
  - No GPSIMD compute; its sequencer issues the bulk SWDGE DMAs (consts,
    rotate-half partition shifts, v roundtrip, output) so the SP hardware
    DMA queue only carries x-tile loads and the v transposes. DMA-queue
    head-of-line blocking dominated v2.
  - No DRAM-roundtrip broadcasts: 1/||q|| and 1/denom are broadcast
    across partitions with a K=1 ones-row matmul into PSUM.
  - 1/||k|| rides into the attention exp as the per-partition activation
    scale; it is computed directly in [tk, tile] layout by making the
    squared-k chunk the matmul stationary and ones the moving operand.
  - All ACT functions stay in the natural_log_exp_and_others table set
    (copy/ln/exp) and the table-load pass is pinned to it — exactly one
    ACT_TABLE_LOAD.
  - Causal masking streams only the valid column suffix of each strip;
    the 128x128 diagonal tile gets a DVE tri-mask.
  - PSUM budget (8 banks): phase A: q(2) k(2) v(1) nq(1) nk(1) bc(1);
    phase C: sc(4) po(2) pd(1) bc(1).
"""

import numpy as np
import ml_dtypes

import concourse.bass as bass
import concourse.tile as tile
from concourse import bacc, mybir
from concourse.bass import ts, ds
from concourse.bass_utils import run_bass_kernel_spmd

# Surface compile-hook exceptions (the PJRT bridge swallows tracebacks).
try:
    import traceback
    import libneuronxla as _lnx

    if not getattr(_lnx, "_err_wrapped", False):
        _orig_cc = _lnx.neuronx_cc

        def _cc_wrapper(*a, **kw):
            try:
                return _orig_cc(*a, **kw)
            except BaseException:
                traceback.print_exc()
                raise

        _lnx.neuronx_cc = _cc_wrapper
        _lnx._err_wrapped = True
except Exception:
    pass

AFT = mybir.ActivationFunctionType
ALU = mybir.AluOpType
F32 = mybir.dt.float32
BF16 = mybir.dt.bfloat16

B, T_FULL, C, D = 8, 2048, 1024, 128
ROPE_BASE = 10000.0
P = 128
TB = 512  # t-block (tq block width, PSUM-bank free dim)
NCO = C // P  # contraction chunks for the QKV projection
H = P // 2


def _pin_act_table():
    """Leave natural_log_exp_and_others as the only candidate activation
    table (it serves every func this kernel uses: copy/ln/exp), so the
    table-load pass emits exactly one ACT_TABLE_LOAD instead of 1.5us
    reloads on every ln<->exp alternation. Positions in the cached dict
    are untouched, so act_func_set_id stays a valid act_info.json index."""
    from concourse.hw_specs import get_activation_tables
    tabs = get_activation_tables("gen3")
    keep = "natural_log_exp_and_others"
    if keep in tabs:
        for name, funcs in tabs.items():
            if name != keep:
                funcs.clear()


def build_nc(T=T_FULL, num_devices=8):
    from contextlib import ExitStack
    _pin_act_table()
    NTB = T // TB
    NKT = T // P
    NC = TB // P  # 128-chunks per block
    nc = bacc.Bacc("TRN2", target_bir_lowering=False, debug=False,
                   num_devices=num_devices)

    xT = nc.dram_tensor("xT", [C, T], BF16, kind="ExternalInput").ap()
    WT = nc.dram_tensor("WT", [C, 3 * D], BF16, kind="ExternalInput").ap()
    cosF = nc.dram_tensor("cosF", [P, T], BF16, kind="ExternalInput").ap()
    sinF = nc.dram_tensor("sinF", [P, T], BF16, kind="ExternalInput").ap()
    tri = nc.dram_tensor("tri", [P, P], BF16, kind="ExternalInput").ap()
    sqk = nc.dram_tensor("sqk", [D, 1], F32, kind="ExternalInput").ap()
    onb = nc.dram_tensor("onb", [P, 1], BF16, kind="ExternalInput").ap()
    onr = nc.dram_tensor("onr", [1, P], BF16, kind="ExternalInput").ap()
    outT = nc.dram_tensor("outT", [D, T], BF16, kind="ExternalOutput").ap()

    xT_t = xT.rearrange("(co p) t -> p co t", p=P)
    WT_t = WT.rearrange("(co p) d -> p co d", p=P)

    with tile.TileContext(nc) as tc:
        with ExitStack() as ctx:
            const = ctx.enter_context(tc.tile_pool(name="const", bufs=1))
            wpool = ctx.enter_context(tc.tile_pool(name="wpool", bufs=3))
            dramp = ctx.enter_context(
                tc.tile_pool(name="dramp", bufs=1, space="DRAM"))

            # wt first on the SP queue (needed by the first matmul);
            # remaining constants go via the gpsimd (SWDGE) queue.
            wt = const.tile([P, NCO, 3 * D], BF16)
            nc.sync.dma_start(wt, WT_t)
            sqk_sb = const.tile([D, 1], F32)
            nc.gpsimd.dma_start(sqk_sb, sqk)
            ones_k = const.tile([P, 1], BF16)
            nc.gpsimd.dma_start(ones_k, onb)
            ones_r = const.tile([1, P], BF16)
            nc.gpsimd.dma_start(ones_r, onr)
            tri_sb = const.tile([P, P], BF16)
            nc.gpsimd.dma_start(tri_sb, tri)
            cos_sb = const.tile([P, T], BF16)
            nc.gpsimd.dma_start(cos_sb, cosF)
            sin_sb = const.tile([P, T], BF16)
            nc.gpsimd.dma_start(sin_sb, sinF)
            # (sqk * C^(1/4))^2 = sqrt(C) * sqk^2 — full logit scale, on q.
            sqk232 = const.tile([D, 1], F32)
            nc.vector.tensor_scalar_mul(sqk232, sqk_sb, float(C ** 0.25))
            nc.vector.tensor_mul(sqk232, sqk232, sqk232)

            qk = const.tile([P, 2 * T], BF16)   # q̃^T | k̃^T (k unnormalized)
            vt = const.tile([P, NKT, P], BF16)  # v tiles [tk, e]
            ink = const.tile([P, NKT], F32)     # 1/||k|| as [tk%P, tile]
            vd = dramp.tile([P, T], BF16)

            # ---------- Phase A: QKV + norms + RoPE (per block) ----------
            with ExitStack() as actx:
                xpool = actx.enter_context(tc.tile_pool(name="xpool", bufs=2))
                ps_qk = actx.enter_context(
                    tc.tile_pool(name="ps_qk", bufs=2, space="PSUM"))
                ps_v = actx.enter_context(
                    tc.tile_pool(name="ps_v", bufs=1, space="PSUM"))
                ps_n = actx.enter_context(
                    tc.tile_pool(name="ps_n", bufs=1, space="PSUM"))
                ps_bc = actx.enter_context(
                    tc.tile_pool(name="ps_bc", bufs=1, space="PSUM"))
                for j in range(NTB):
                    tsl = ds(j * TB, TB)
                    with nc.named_scope(f"qkv{j}"):
                        xts = []
                        for half in range(2):
                            xt = xpool.tile([P, NCO // 2, TB], BF16,
                                            tag=f"xt{half}")
                            nc.sync.dma_start(
                                xt, xT_t[:, ds(half * NCO // 2, NCO // 2),
                                         tsl])
                            xts.append(xt)
                        psq = ps_qk.tile([P, TB], F32, tag="q")
                        psk = ps_qk.tile([P, TB], F32, tag="k")
                        psv = ps_v.tile([P, TB], F32, tag="v")
                        for g, pg in ((0, psq), (1, psk), (2, psv)):
                            for co in range(NCO):
                                nc.tensor.matmul(
                                    pg, wt[:, co, ts(g, D)],
                                    xts[co // 4][:, co % 4, :],
                                    start=(co == 0), stop=(co == NCO - 1))
                        qkraw = wpool.tile([P, 2, TB], BF16, tag="qkraw")
                        nc.scalar.activation(qkraw[:, 0, :], psq, AFT.Copy)
                        nc.scalar.activation(qkraw[:, 1, :], psk, AFT.Copy)
                        vst = wpool.tile([P, TB], BF16, tag="vst")
                        nc.vector.tensor_copy(vst, psv)

                    with nc.named_scope(f"norm{j}"):
                        sq = wpool.tile([P, 2, TB], BF16, tag="sq")
                        nc.vector.tensor_mul(sq[:, 0, :], qkraw[:, 0, :],
                                             qkraw[:, 0, :])
                        nc.vector.tensor_mul(sq[:, 1, :], qkraw[:, 1, :],
                                             qkraw[:, 1, :])
                        # q: row layout [1, TB] (ones stationary), for the
                        # PE partition-broadcast below.
                        nq = ps_n.tile([1, TB], F32, tag="nq")
                        nc.tensor.matmul(nq, ones_k, sq[:, 0, :],
                                         start=True, stop=True)
                        lnq = wpool.tile([1, TB], F32, tag="lnq")
                        nc.scalar.activation(lnq, nq, AFT.Ln)
                        invq = wpool.tile([1, TB], BF16, tag="invq")
                        nc.scalar.activation(invq, lnq, AFT.Exp, scale=-0.5)
                        bcq = ps_bc.tile([P, TB], F32, tag="bcq")
                        nc.tensor.matmul(bcq, ones_r, invq,
                                         start=True, stop=True)
                        # k: [tk, tile] layout (sq chunk stationary), feeds
                        # the attention exp scale directly — no transpose.
                        nk = ps_n.tile([P, NC], F32, tag="nk")
                        for c in range(NC):
                            nc.tensor.matmul(
                                nk[:, ds(c, 1)], sq[:, 1, ts(c, P)],
                                ones_k, start=True, stop=True)
                        lnk = wpool.tile([P, NC], F32, tag="lnk")
                        nc.scalar.activation(lnk, nk, AFT.Ln)
                        nc.scalar.activation(ink[:, ds(j * NC, NC)], lnk,
                                             AFT.Exp, scale=-0.5)

                    with nc.named_scope(f"rope{j}"):
                        # rotate_half via partition-shift DMAs (sign folded
                        # into the sin table); q and k in one shot.
                        rot = wpool.tile([P, 2, TB], BF16, tag="rot")
                        nc.gpsimd.dma_start(rot[0:H, :, :],
                                            qkraw[H:P, :, :])
                        nc.gpsimd.dma_start(rot[H:P, :, :],
                                            qkraw[0:H, :, :])

                        m1 = wpool.tile([P, TB], BF16, tag="m1")
                        nc.vector.tensor_mul(m1, qkraw[:, 0, :],
                                             cos_sb[:, tsl])
                        m2 = wpool.tile([P, TB], BF16, tag="m2")
                        nc.vector.tensor_mul(m2, rot[:, 0, :],
                                             sin_sb[:, tsl])
                        m12 = wpool.tile([P, TB], BF16, tag="m12")
                        nc.vector.tensor_add(m12, m1, m2)
                        nc.vector.scalar_tensor_tensor(
                            out=qk[:, tsl], in0=m12, scalar=sqk232,
                            in1=bcq, op0=ALU.mult, op1=ALU.mult)

                        m1k = wpool.tile([P, TB], BF16, tag="m1k")
                        nc.vector.tensor_mul(m1k, qkraw[:, 1, :],
                                             cos_sb[:, tsl])
                        m2k = wpool.tile([P, TB], BF16, tag="m2k")
                        nc.vector.tensor_mul(m2k, rot[:, 1, :],
                                             sin_sb[:, tsl])
                        nc.vector.tensor_add(qk[:, ds(T + j * TB, TB)],
                                             m1k, m2k)

                        # v transpose via DRAM-roundtrip XBAR DMA (bf16)
                        nc.gpsimd.dma_start(vd[:, tsl], vst)
                        for i in range(4 * j, 4 * j + 4):
                            nc.sync.dma_start_transpose(vt[:, i, :],
                                                        vd[:, ts(i, P)])

            # ---------- Phase C: causal attention ----------
            with ExitStack() as cctx:
                expool = cctx.enter_context(
                    tc.tile_pool(name="expool", bufs=4))
                ps_sc = cctx.enter_context(
                    tc.tile_pool(name="ps_sc", bufs=4, space="PSUM"))
                ps_o = cctx.enter_context(
                    tc.tile_pool(name="ps_o", bufs=2, space="PSUM"))
                ps_d = cctx.enter_context(
                    tc.tile_pool(name="ps_d", bufs=1, space="PSUM"))
                ps_b2 = cctx.enter_context(
                    tc.tile_pool(name="ps_b2", bufs=1, space="PSUM"))

                for J in range(NTB):
                    with nc.named_scope(f"att{J}"):
                        q_blk = qk[:, ts(J, TB)]
                        po = ps_o.tile([P, TB], F32, tag="o")
                        pd = ps_d.tile([1, TB], F32, tag="d")
                        nstr = (TB // P) * (J + 1)
                        for i in range(nstr):
                            dr = i - (TB // P) * J
                            off = P * dr if dr >= 0 else 0
                            w = TB - off
                            sc = ps_sc.tile([P, TB], F32, tag="sc")
                            nc.tensor.matmul(
                                sc[:, ds(off, w)], qk[:, ds(T + P * i, P)],
                                q_blk[:, ds(off, w)], start=True, stop=True)
                            ex = expool.tile([P, TB], BF16, tag="ex")
                            nc.scalar.activation(
                                ex[:, ds(off, w)], sc[:, ds(off, w)],
                                AFT.Exp, scale=ink[:, i:i + 1])
                            if dr >= 0:
                                nc.vector.tensor_mul(
                                    ex[:, ds(off, P)], ex[:, ds(off, P)],
                                    tri_sb)
                            nc.tensor.matmul(
                                po[:, ds(off, w)], vt[:, i, :],
                                ex[:, ds(off, w)],
                                start=(i == 0), stop=(i == nstr - 1))
                            nc.tensor.matmul(
                                pd[:, ds(off, w)], ones_k,
                                ex[:, ds(off, w)],
                                start=(i == 0), stop=(i == nstr - 1))

                        lnd = wpool.tile([1, TB], F32, tag="lnd")
                        nc.scalar.activation(lnd, pd, AFT.Ln)
                        invd = wpool.tile([1, TB], BF16, tag="invd")
                        nc.scalar.activation(invd, lnd, AFT.Exp, scale=-1.0)
                        bcd = ps_b2.tile([P, TB], F32, tag="bcd")
                        nc.tensor.matmul(bcd, ones_r, invd,
                                         start=True, stop=True)
                        bcs = wpool.tile([P, TB], BF16, tag="bcs")
                        nc.vector.tensor_copy(bcs, bcd)
                        ob = wpool.tile([P, TB], BF16, tag="ob")
                        nc.vector.tensor_mul(ob, po, bcs)
                        nc.gpsimd.dma_start(outT[:, ts(J, TB)], ob)

    nc.compile()
    return nc


def _host_tables(T):
    d = D
    inv_freq = 1.0 / (ROPE_BASE ** (np.arange(0, d, 2, dtype=np.float64) / d))
    t = np.arange(T, dtype=np.float64)
    freqs = np.outer(inv_freq, t)  # [d/2, T]
    emb = np.concatenate([freqs, freqs], axis=0)  # [d, T]
    cos1 = np.cos(emb)
    sin1 = np.sin(emb)
    # sign of rotate_half folded into the table: rot is built with plain
    # copies, and sin rows 0:d/2 carry the minus sign instead.
    sin1[: d // 2, :] *= -1.0
    cosF = np.ascontiguousarray(cos1).astype(ml_dtypes.bfloat16)
    sinF = np.ascontiguousarray(sin1).astype(ml_dtypes.bfloat16)
    a = np.arange(P)
    tri = (a[None, :] >= a[:, None]).astype(ml_dtypes.bfloat16)  # [tk, tq]
    return cosF, sinF, tri


TRACE = False
LAST_EXEC_NS = None
LAST_TRACE = None
LAST_INSTS = None


def kernel(x, W_qkv, sqk):
    global LAST_EXEC_NS, LAST_TRACE, LAST_INSTS
    T = x.shape[1]
    cosF, sinF, tri = _host_tables(T)
    WT = np.ascontiguousarray(np.asarray(W_qkv).T).astype(ml_dtypes.bfloat16)
    sqk2 = np.ascontiguousarray(
        np.asarray(sqk).reshape(D, 1)).astype(np.float32)
    in_maps = []
    for b in range(B):
        in_maps.append({
            "xT": np.ascontiguousarray(
                np.asarray(x[b]).T).astype(ml_dtypes.bfloat16),
            "WT": WT,
            "cosF": cosF,
            "sinF": sinF,
            "tri": tri,
            "sqk": sqk2,
            "onb": np.ones((P, 1), ml_dtypes.bfloat16),
            "onr": np.ones((1, P), ml_dtypes.bfloat16),
        })
    nc = build_nc(T=T, num_devices=B)
    res = run_bass_kernel_spmd(nc, in_maps, core_ids=list(range(B)),
                               trace=TRACE)
    LAST_EXEC_NS = res.exec_time_ns
    LAST_TRACE = (res.instructions_and_trace[1]
                  if res.instructions_and_trace else None)
    LAST_INSTS = (res.instructions_and_trace[0]
                  if res.instructions_and_trace else None)
    out = np.stack([r["outT"].T for r in res.results])  # [B, T, D]
    return np.ascontiguousarray(out).astype(np.float32)
